# revision 8
# baseline (speedup 1.0000x reference)
"""Trainium2 Bass kernel for nn_CorrelationLayer (441-displacement cost volume).

result[k, i, j] = sum_c f1[c, i, j] * pad(f2)[c, i + dy_k, j + dx_k]
with (dy, dx) in {0, 2, ..., 40}^2, H, W = 48, 64, C = 128, pad D = 20.

Strategy (v2: column-parity split)
---------------------------------
Displacements are stride-2 in both axes, so f1 column j only ever
correlates with f2 columns of the SAME parity.  Splitting columns by
parity turns the per-row-pair all-pairs matrix from 64x64 into two
32x32 blocks: half the TensorE work, half the PSUM->SBUF copy traffic
and half the output DMA vs. the unsplit scheme.

Sharding: 8 cores = (row parity rp) x (col parity cp) x (half h).
Each core holds 12 f2 rows of parity rp restricted to cp columns
(3 stationary tiles of 4 rows x 32 cols = 128) and all 24 f1 rows of
parity rp at cp columns (768 moving cols).  Three [128,128]x[128,768]
matmuls produce M[(rq,v), (s,u)] = sum_c f2[c,R[4t+rq],J[v]] *
f1[c,I[s],J[u]].  The band/diagonal gather and zero-padding are a pure
data rearrangement done on host during unsharding -- all arithmetic
happens on device.
"""

import sys
import types

for _p in ("/opt/trn_rl_repo", "/root/.axon_site"):
    if _p not in sys.path:
        sys.path.insert(0, _p)

import ml_dtypes
import numpy as np

BF16 = ml_dtypes.bfloat16

import concourse.bacc as bacc
import concourse.mybir as mybir
from concourse import tile
from concourse import bass_utils
from concourse.bass_utils import run_bass_kernel_spmd

C = 128
H = 48
W = 64
D = 20
ND = 21          # displacements per axis
NCORES = 8
NT = 3           # stationary tiles per core (4 f2 rows x 32 cols each)
MOV = 24 * 32    # 768 moving columns (24 f1 rows x 32 same-parity cols)
WARM_N = 6       # PE warm-up matmuls
WARM_COLS = 512
PSUM_BF16 = False  # matmul output must be fp32 (bass asserts)


def _ensure_ntff_hook():
    """Register the axon NTFF profile hook if possible (for trace runs)."""
    try:
        import antenv
        if "antenv.axon_hooks" not in sys.modules:
            mod = types.ModuleType("antenv.axon_hooks")
            _h = [None]
            mod.set_axon_ntff_profile_hook = lambda h: _h.__setitem__(0, h)
            mod.get_axon_ntff_profile_hook = lambda: _h[0]
            sys.modules["antenv.axon_hooks"] = mod
            antenv.axon_hooks = mod
        bass_utils.upload_artifacts = lambda tmpdir: "local://" + tmpdir
        from trn_agent_boot.trn_boot import _ntff_profile_via_ctypes
        sys.modules["antenv.axon_hooks"].set_axon_ntff_profile_hook(
            _ntff_profile_via_ctypes("/opt/axon/libaxon_pjrt.so")
        )
    except Exception:
        pass


def build_program():
    nc = bacc.Bacc(None, target_bir_lowering=False)
    f1g = nc.declare_dram_parameter("f1g", [C, MOV], mybir.dt.bfloat16, isOutput=False)
    f2g = nc.declare_dram_parameter("f2g", [C, NT * 128], mybir.dt.bfloat16, isOutput=False)
    mout = nc.declare_dram_parameter(
        "mout", [C, NT * MOV], mybir.dt.bfloat16, isOutput=True
    )

    with tile.TileContext(nc) as tc:
        with (
            tc.tile_pool(name="in", bufs=1) as in_pool,
            tc.tile_pool(name="msb", bufs=3) as m_pool,
            tc.tile_pool(name="psw", bufs=1, space="PSUM") as psw_pool,
            tc.tile_pool(name="ps", bufs=3, space="PSUM") as ps_pool,
        ):
            # inputs: f2 (stationary) on the SP ring, f1 split across both
            # rings so the first matmul can start as early as possible
            f2_sb = in_pool.tile([C, NT * 128], mybir.dt.bfloat16)
            nc.sync.dma_start(out=f2_sb[:], in_=f2g[:])
            f1a = in_pool.tile([C, 512], mybir.dt.bfloat16, tag="f1a")
            nc.scalar.dma_start(out=f1a[:], in_=f1g[:, :512])
            f1b = in_pool.tile([C, 256], mybir.dt.bfloat16, tag="f1b")
            nc.sync.dma_start(out=f1b[:], in_=f1g[:, 512:])

            # PE warm-up: dependency-free matmuls on scratch keep the PE busy
            # while input DMAs are in flight so the HAM clock gate ramps up
            scratch = in_pool.tile([C, WARM_COLS], mybir.dt.bfloat16, tag="scratch")
            nc.gpsimd.memset(scratch[:], 0)
            ps_warm = psw_pool.tile([128, MOV], mybir.dt.float32, tag="psw")
            for _ in range(WARM_N):
                nc.tensor.matmul(
                    ps_warm[:, :WARM_COLS], scratch[:, :128], scratch[:],
                    start=True, stop=True,
                )

            ps_dt = mybir.dt.bfloat16 if PSUM_BF16 else mybir.dt.float32
            for t in range(NT):
                ps = ps_pool.tile([128, MOV], ps_dt, tag="ps")
                lhsT = f2_sb[:, 128 * t : 128 * (t + 1)]
                nc.tensor.matmul(ps[:, :512], lhsT, f1a[:], start=True, stop=True)
                nc.tensor.matmul(ps[:, 512:], lhsT, f1b[:], start=True, stop=True)
                m_sb = m_pool.tile([128, MOV], mybir.dt.bfloat16, tag=f"m{t}")
                if t == 1:
                    nc.scalar.copy(m_sb[:], ps[:])
                else:
                    nc.vector.tensor_copy(m_sb[:], ps[:])
                lane = nc.scalar if t == 1 else nc.sync
                lane.dma_start(out=mout[:, MOV * t : MOV * (t + 1)], in_=m_sb[:])
    nc.compile()
    return nc


_PROGRAM_CACHE = {}


def _get_program():
    if "nc" not in _PROGRAM_CACHE:
        _PROGRAM_CACHE["nc"] = build_program()
    return _PROGRAM_CACHE["nc"]


def _core_rows(m):
    """Core m = rp*4 + cp*2 + h -> (rp, cp, f2 rows R, f1 rows I, cols J)."""
    rp, cp, h = m // 4, (m // 2) % 2, m % 2
    R = [rp + 2 * (12 * h + t) for t in range(12)]
    I = [rp + 2 * s for s in range(24)]
    J = [cp + 2 * u for u in range(32)]
    return rp, cp, h, R, I, J


def _shard_inputs(features_1, features_2):
    f1 = np.ascontiguousarray(features_1, dtype=np.float32)
    f2 = np.ascontiguousarray(features_2, dtype=np.float32)
    in_maps = []
    for m in range(NCORES):
        rp, cp, h, R, I, J = _core_rows(m)
        f2g = f2[:, R][:, :, J].reshape(C, NT * 128)
        f1g = f1[:, I][:, :, J].reshape(C, MOV)
        in_maps.append(
            {
                "f1g": np.ascontiguousarray(f1g).astype(BF16),
                "f2g": np.ascontiguousarray(f2g).astype(BF16),
            }
        )
    return in_maps


def _assemble(results):
    """Gather the stride-2 displacement band out of the per-core all-pairs
    blocks (pure indexing -- no arithmetic)."""
    M = np.stack(
        [np.asarray(results[m]["mout"]).astype(np.float32) for m in range(NCORES)]
    )  # [8, 128, 2304]

    dy, dxi, i, j = np.ogrid[0:ND, 0:ND, 0:H, 0:W]
    rp = i & 1
    cp = j & 1
    u = j >> 1
    r2 = i + 2 * dy - 20
    v = u + dxi - 10
    valid = (r2 >= 0) & (r2 < H) & (v >= 0) & (v < 32)
    r2c = np.clip(r2, 0, H - 1)
    vc = np.clip(v, 0, 31)
    k = (r2c - rp) >> 1
    h = k // 12
    l = k % 12
    t = l // 4
    rq = l % 4
    s = (i - rp) >> 1
    m = rp * 4 + cp * 2 + h
    part = 32 * rq + vc
    col = MOV * t + 32 * s + u
    bm, bp, bc = np.broadcast_arrays(m, part, col)
    out = M[bm, bp, bc]
    out[~np.broadcast_to(valid, out.shape)] = 0.0
    return out.reshape(1, ND * ND, H, W)


def kernel(features_1, features_2):
    nc = _get_program()
    in_maps = _shard_inputs(features_1, features_2)
    res = run_bass_kernel_spmd(nc, in_maps, list(range(NCORES)))
    return _assemble(res.results)


def kernel_traced(features_1, features_2, tmpdir=None):
    """Same as kernel() but with NTFF profiling; returns (output, exec_time_ns)."""
    _ensure_ntff_hook()
    nc = _get_program()
    in_maps = _shard_inputs(features_1, features_2)
    res = run_bass_kernel_spmd(
        nc, in_maps, list(range(NCORES)), trace=True, tmpdir=tmpdir
    )
    return _assemble(res.results), res.exec_time_ns


# revision 9
# speedup vs baseline: 1.0409x; 1.0409x over previous
"""Trainium2 Bass kernel for nn_CorrelationLayer (441-displacement cost volume).

result[k, i, j] = sum_c f1[c, i, j] * pad(f2)[c, i + dy_k, j + dx_k]
with (dy, dx) in {0, 2, ..., 40}^2, H, W = 48, 64, C = 128, pad D = 20.

Strategy (v3: column-parity split + valid-window trimming)
----------------------------------------------------------
Displacements are stride-2 in both axes, so f1 column j only ever
correlates with f2 columns of the SAME parity: the per-row-pair
all-pairs block is 32x32 per parity instead of 64x64 (2x less PE /
copy / DMA than the unsplit scheme).

Sharding: 8 cores = (row parity rp) x (col parity cp) x (half h).
Each core holds 12 f2 rows of parity rp at cp columns, as 3 stationary
tiles ("slots") of 4 rows x 32 cols.  The y-displacement window |r2-i|
<= 20 means slot quads near the volume edge need only 14/18/22 of the
24 same-parity f1 rows; with h=1 cores taking mirrored quads (and f1
rows stored reversed), the slot windows are uniformly 448/576/704
moving columns across all cores -- a single SPMD program computes only
the needed band (1728 instead of 2304 columns).

The band/diagonal gather and zero-fill is a pure data rearrangement
done on host during unsharding -- all arithmetic happens on device.
"""

import sys
import types

for _p in ("/opt/trn_rl_repo", "/root/.axon_site"):
    if _p not in sys.path:
        sys.path.insert(0, _p)

import ml_dtypes
import numpy as np

BF16 = ml_dtypes.bfloat16

import concourse.bacc as bacc
import concourse.mybir as mybir
from concourse import tile
from concourse import bass_utils
from concourse.bass_utils import run_bass_kernel_spmd

C = 128
H = 48
W = 64
D = 20
ND = 21            # displacements per axis
NCORES = 8
SLOT_COLS = (448, 576, 704)   # moving cols per stationary slot (A, B, C)
SLOT_OFF = (0, 448, 1024)
TOT = 1728                    # total output columns per core
F1COLS = 704                  # f1 moving columns per core (22 rows x 32)
WARM_N = 6                    # PE warm-up matmuls
WARM_COLS = 512


def _ensure_ntff_hook():
    """Register the axon NTFF profile hook if possible (for trace runs)."""
    try:
        import antenv
        if "antenv.axon_hooks" not in sys.modules:
            mod = types.ModuleType("antenv.axon_hooks")
            _h = [None]
            mod.set_axon_ntff_profile_hook = lambda h: _h.__setitem__(0, h)
            mod.get_axon_ntff_profile_hook = lambda: _h[0]
            sys.modules["antenv.axon_hooks"] = mod
            antenv.axon_hooks = mod
        bass_utils.upload_artifacts = lambda tmpdir: "local://" + tmpdir
        from trn_agent_boot.trn_boot import _ntff_profile_via_ctypes
        sys.modules["antenv.axon_hooks"].set_axon_ntff_profile_hook(
            _ntff_profile_via_ctypes("/opt/axon/libaxon_pjrt.so")
        )
    except Exception:
        pass


def build_program():
    nc = bacc.Bacc(None, target_bir_lowering=False)
    # aux = f2 stationary (384 cols) ++ f1 tail [512:704] (192 cols)
    aux = nc.declare_dram_parameter("aux", [C, 576], mybir.dt.bfloat16, isOutput=False)
    f1a = nc.declare_dram_parameter("f1a", [C, 512], mybir.dt.bfloat16, isOutput=False)
    mout = nc.declare_dram_parameter("mout", [C, TOT], mybir.dt.bfloat16, isOutput=True)

    with tile.TileContext(nc) as tc:
        with (
            tc.tile_pool(name="in", bufs=1) as in_pool,
            tc.tile_pool(name="msb", bufs=1) as m_pool,
            tc.tile_pool(name="psw", bufs=1, space="PSUM") as psw_pool,
            tc.tile_pool(name="ps", bufs=1, space="PSUM") as ps_pool,
        ):
            aux_sb = in_pool.tile([C, 576], mybir.dt.bfloat16, tag="aux")
            nc.sync.dma_start(out=aux_sb[:], in_=aux[:])
            f1a_sb = in_pool.tile([C, 512], mybir.dt.bfloat16, tag="f1a")
            nc.scalar.dma_start(out=f1a_sb[:], in_=f1a[:])

            # PE warm-up: dependency-free matmuls on scratch keep the PE busy
            # while input DMAs are in flight so the HAM clock gate ramps up
            scratch = in_pool.tile([C, WARM_COLS], mybir.dt.bfloat16, tag="scratch")
            nc.gpsimd.memset(scratch[:], 0)
            ps_warm = psw_pool.tile([128, WARM_COLS], mybir.dt.float32, tag="psw")
            for _ in range(WARM_N):
                nc.tensor.matmul(
                    ps_warm[:, :WARM_COLS], scratch[:, :128], scratch[:],
                    start=True, stop=True,
                )

            def lhsT(x):
                return aux_sb[:, 128 * x : 128 * (x + 1)]

            f1b_sb = aux_sb  # f1 cols [512:704] live at aux cols [384:576]

            psA = ps_pool.tile([128, 448], mybir.dt.float32, tag="psA")
            psB = ps_pool.tile([128, 576], mybir.dt.float32, tag="psB")
            psC = ps_pool.tile([128, 704], mybir.dt.float32, tag="psC")

            # first-chunk matmuls gated only on f1a (+aux for stationary);
            # A completes first so its cast/DMA overlap the rest
            nc.tensor.matmul(psA[:, :448], lhsT(0), f1a_sb[:, :448], start=True, stop=True)
            mA = m_pool.tile([128, 448], mybir.dt.bfloat16, tag="mA")
            nc.vector.tensor_copy(mA[:], psA[:])
            nc.sync.dma_start(out=mout[:, 0:448], in_=mA[:])

            nc.tensor.matmul(psC[:, :512], lhsT(2), f1a_sb[:], start=True, stop=True)
            nc.tensor.matmul(psB[:, :512], lhsT(1), f1a_sb[:], start=True, stop=True)
            nc.tensor.matmul(
                psC[:, 512:704], lhsT(2), f1b_sb[:, 384:576], start=True, stop=True
            )
            mC = m_pool.tile([128, 704], mybir.dt.bfloat16, tag="mC")
            nc.vector.tensor_copy(mC[:], psC[:])
            nc.scalar.dma_start(out=mout[:, 1024:1728], in_=mC[:])

            nc.tensor.matmul(
                psB[:, 512:576], lhsT(1), f1b_sb[:, 384:448], start=True, stop=True
            )
            mB = m_pool.tile([128, 576], mybir.dt.bfloat16, tag="mB")
            nc.scalar.copy(mB[:], psB[:])
            nc.sync.dma_start(out=mout[:, 448:1024], in_=mB[:])
    nc.compile()
    return nc


_PROGRAM_CACHE = {}


def _get_program():
    if "nc" not in _PROGRAM_CACHE:
        _PROGRAM_CACHE["nc"] = build_program()
    return _PROGRAM_CACHE["nc"]


def _core_def(m):
    """Core m = rp*4 + cp*2 + h -> (rp, cp, h, f2 rows R, f1 rows I, cols J)."""
    rp, cp, h = m // 4, (m // 2) % 2, m % 2
    gs = (0, 1, 2) if h == 0 else (5, 4, 3)
    R = [rp + 8 * g + 2 * rq for g in gs for rq in range(4)]
    S = range(0, 22) if h == 0 else range(23, 1, -1)
    I = [rp + 2 * s for s in S]
    J = [cp + 2 * u for u in range(32)]
    return rp, cp, h, R, I, J


def _shard_inputs(features_1, features_2):
    f1 = np.ascontiguousarray(features_1, dtype=np.float32)
    f2 = np.ascontiguousarray(features_2, dtype=np.float32)
    in_maps = []
    for m in range(NCORES):
        rp, cp, h, R, I, J = _core_def(m)
        f2g = f2[:, R][:, :, J].reshape(C, 384)
        f1g = f1[:, I][:, :, J].reshape(C, F1COLS)
        aux = np.concatenate([f2g, f1g[:, 512:]], axis=1)
        in_maps.append(
            {
                "aux": np.ascontiguousarray(aux).astype(BF16),
                "f1a": np.ascontiguousarray(f1g[:, :512]).astype(BF16),
            }
        )
    return in_maps


def _assemble(results):
    """Gather the stride-2 displacement band out of the per-core blocks
    (pure indexing -- no arithmetic)."""
    M = np.stack(
        [np.asarray(results[m]["mout"]).astype(np.float32) for m in range(NCORES)]
    )  # [8, 128, 1728]

    dy, dxi, i, j = np.ogrid[0:ND, 0:ND, 0:H, 0:W]
    rp = i & 1
    cp = j & 1
    u = j >> 1
    r2 = i + 2 * dy - 20
    v = u + dxi - 10
    valid = (r2 >= 0) & (r2 < H) & (v >= 0) & (v < 32)
    r2c = np.clip(r2, 0, H - 1)
    vc = np.clip(v, 0, 31)
    k = (r2c - rp) >> 1
    g = k // 4
    rq = k % 4
    h = (g >= 3).astype(int)
    x = np.where(h == 0, g, 5 - g)
    s = (i - rp) >> 1
    l = np.where(h == 0, s, 23 - s)
    off = np.array(SLOT_OFF)[x]
    m = rp * 4 + cp * 2 + h
    part = 32 * rq + vc
    col = off + 32 * l + u
    bm, bp, bc = np.broadcast_arrays(m, part, col)
    out = M[bm, bp, bc]
    out[~np.broadcast_to(valid, out.shape)] = 0.0
    return out.reshape(1, ND * ND, H, W)


def kernel(features_1, features_2):
    nc = _get_program()
    in_maps = _shard_inputs(features_1, features_2)
    res = run_bass_kernel_spmd(nc, in_maps, list(range(NCORES)))
    return _assemble(res.results)


def kernel_traced(features_1, features_2, tmpdir=None):
    """Same as kernel() but with NTFF profiling; returns (output, exec_time_ns)."""
    _ensure_ntff_hook()
    nc = _get_program()
    in_maps = _shard_inputs(features_1, features_2)
    res = run_bass_kernel_spmd(
        nc, in_maps, list(range(NCORES)), trace=True, tmpdir=tmpdir
    )
    return _assemble(res.results), res.exec_time_ns


# revision 12
# speedup vs baseline: 1.0530x; 1.0116x over previous
"""Trainium2 Bass kernel for nn_CorrelationLayer (441-displacement cost volume).

result[k, i, j] = sum_c f1[c, i, j] * pad(f2)[c, i + dy_k, j + dx_k]
with (dy, dx) in {0, 2, ..., 40}^2, H, W = 48, 64, C = 128, pad D = 20.

Strategy (v3: column-parity split + valid-window trimming)
----------------------------------------------------------
Displacements are stride-2 in both axes, so f1 column j only ever
correlates with f2 columns of the SAME parity: the per-row-pair
all-pairs block is 32x32 per parity instead of 64x64 (2x less PE /
copy / DMA than the unsplit scheme).

Sharding: 8 cores = (row parity rp) x (col parity cp) x (half h).
Each core holds 12 f2 rows of parity rp at cp columns, as 3 stationary
tiles ("slots") of 4 rows x 32 cols.  The y-displacement window |r2-i|
<= 20 means slot quads near the volume edge need only 14/18/22 of the
24 same-parity f1 rows; with h=1 cores taking mirrored quads (and f1
rows stored reversed), the slot windows are uniformly 448/576/704
moving columns across all cores -- a single SPMD program computes only
the needed band (1728 instead of 2304 columns).

The band/diagonal gather and zero-fill is a pure data rearrangement
done on host during unsharding -- all arithmetic happens on device.
"""

import sys
import types

for _p in ("/opt/trn_rl_repo", "/root/.axon_site"):
    if _p not in sys.path:
        sys.path.insert(0, _p)

import ml_dtypes
import numpy as np

BF16 = ml_dtypes.bfloat16

import concourse.bacc as bacc
import concourse.mybir as mybir
from concourse import tile
from concourse import bass_utils
from concourse.bass_utils import run_bass_kernel_spmd

C = 128
H = 48
W = 64
D = 20
ND = 21            # displacements per axis
NCORES = 8
SLOT_COLS = (448, 576, 704)   # moving cols per stationary slot (A, B, C)
SLOT_OFF = (0, 448, 1024)
TOT = 1728                    # total output columns per core
F1COLS = 704                  # f1 moving columns per core (22 rows x 32)
WARM_N = 4                    # PE warm-up matmuls
WARM_COLS = 512


def _ensure_ntff_hook():
    """Register the axon NTFF profile hook if possible (for trace runs)."""
    try:
        import antenv
        if "antenv.axon_hooks" not in sys.modules:
            mod = types.ModuleType("antenv.axon_hooks")
            _h = [None]
            mod.set_axon_ntff_profile_hook = lambda h: _h.__setitem__(0, h)
            mod.get_axon_ntff_profile_hook = lambda: _h[0]
            sys.modules["antenv.axon_hooks"] = mod
            antenv.axon_hooks = mod
        bass_utils.upload_artifacts = lambda tmpdir: "local://" + tmpdir
        from trn_agent_boot.trn_boot import _ntff_profile_via_ctypes
        sys.modules["antenv.axon_hooks"].set_axon_ntff_profile_hook(
            _ntff_profile_via_ctypes("/opt/axon/libaxon_pjrt.so")
        )
    except Exception:
        pass


def build_program():
    nc = bacc.Bacc(None, target_bir_lowering=False)
    # aux = f2 stationary (384 cols) ++ f1 tail [512:704] (192 cols)
    aux = nc.declare_dram_parameter("aux", [C, 576], mybir.dt.bfloat16, isOutput=False)
    f1a = nc.declare_dram_parameter("f1a", [C, 512], mybir.dt.bfloat16, isOutput=False)
    mout = nc.declare_dram_parameter("mout", [C, TOT], mybir.dt.bfloat16, isOutput=True)

    with tile.TileContext(nc) as tc:
        with (
            tc.tile_pool(name="in", bufs=1) as in_pool,
            tc.tile_pool(name="msb", bufs=1) as m_pool,
            tc.tile_pool(name="psw", bufs=1, space="PSUM") as psw_pool,
            tc.tile_pool(name="ps", bufs=1, space="PSUM") as ps_pool,
        ):
            # f1a gates every matmul -> put it on the SP ring (starts
            # earliest); aux (stationary + f1 tail) is needed later
            f1a_sb = in_pool.tile([C, 512], mybir.dt.bfloat16, tag="f1a")
            nc.sync.dma_start(out=f1a_sb[:], in_=f1a[:])
            aux_sb = in_pool.tile([C, 576], mybir.dt.bfloat16, tag="aux")
            nc.scalar.dma_start(out=aux_sb[:], in_=aux[:])

            # PE warm-up: dependency-free matmuls on scratch keep the PE busy
            # while input DMAs are in flight so the HAM clock gate ramps up
            scratch = in_pool.tile([C, WARM_COLS], mybir.dt.bfloat16, tag="scratch")
            nc.gpsimd.memset(scratch[:], 0)
            ps_warm = psw_pool.tile([128, WARM_COLS], mybir.dt.float32, tag="psw")
            for _ in range(WARM_N):
                nc.tensor.matmul(
                    ps_warm[:, :WARM_COLS], scratch[:, :128], scratch[:],
                    start=True, stop=True,
                )

            def lhsT(x):
                return aux_sb[:, 128 * x : 128 * (x + 1)]

            f1b_sb = aux_sb  # f1 cols [512:704] live at aux cols [384:576]

            psA = ps_pool.tile([128, 448], mybir.dt.float32, tag="psA")
            psB = ps_pool.tile([128, 576], mybir.dt.float32, tag="psB")
            psC = ps_pool.tile([128, 704], mybir.dt.float32, tag="psC")

            # first-chunk matmuls gated only on f1a (+aux for stationary);
            # A completes first so its cast/DMA overlap the rest
            nc.tensor.matmul(psA[:, :448], lhsT(0), f1a_sb[:, :448], start=True, stop=True)
            mA = m_pool.tile([128, 448], mybir.dt.bfloat16, tag="mA")
            nc.vector.tensor_copy(mA[:], psA[:])
            nc.sync.dma_start(out=mout[:, 0:448], in_=mA[:])

            nc.tensor.matmul(psC[:, :512], lhsT(2), f1a_sb[:], start=True, stop=True)
            nc.tensor.matmul(
                psC[:, 512:704], lhsT(2), f1b_sb[:, 384:576], start=True, stop=True
            )
            mC = m_pool.tile([128, 704], mybir.dt.bfloat16, tag="mC")
            nc.vector.tensor_copy(mC[:], psC[:])
            nc.sync.dma_start(out=mout[:, 1024:1728], in_=mC[:])

            nc.tensor.matmul(psB[:, :512], lhsT(1), f1a_sb[:], start=True, stop=True)
            nc.tensor.matmul(
                psB[:, 512:576], lhsT(1), f1b_sb[:, 384:448], start=True, stop=True
            )
            mB = m_pool.tile([128, 576], mybir.dt.bfloat16, tag="mB")
            nc.scalar.copy(mB[:], psB[:])
            nc.scalar.dma_start(out=mout[:, 448:1024], in_=mB[:])
    nc.compile()
    return nc


_PROGRAM_CACHE = {}


def _get_program():
    if "nc" not in _PROGRAM_CACHE:
        _PROGRAM_CACHE["nc"] = build_program()
    return _PROGRAM_CACHE["nc"]


def _core_def(m):
    """Core m = rp*4 + cp*2 + h -> (rp, cp, h, f2 rows R, f1 rows I, cols J)."""
    rp, cp, h = m // 4, (m // 2) % 2, m % 2
    gs = (0, 1, 2) if h == 0 else (5, 4, 3)
    R = [rp + 8 * g + 2 * rq for g in gs for rq in range(4)]
    S = range(0, 22) if h == 0 else range(23, 1, -1)
    I = [rp + 2 * s for s in S]
    J = [cp + 2 * u for u in range(32)]
    return rp, cp, h, R, I, J


def _shard_inputs(features_1, features_2):
    f1 = np.ascontiguousarray(features_1, dtype=np.float32)
    f2 = np.ascontiguousarray(features_2, dtype=np.float32)
    in_maps = []
    for m in range(NCORES):
        rp, cp, h, R, I, J = _core_def(m)
        f2g = f2[:, R][:, :, J].reshape(C, 384)
        f1g = f1[:, I][:, :, J].reshape(C, F1COLS)
        aux = np.concatenate([f2g, f1g[:, 512:]], axis=1)
        in_maps.append(
            {
                "aux": np.ascontiguousarray(aux).astype(BF16),
                "f1a": np.ascontiguousarray(f1g[:, :512]).astype(BF16),
            }
        )
    return in_maps


def _assemble(results):
    """Gather the stride-2 displacement band out of the per-core blocks
    (pure indexing -- no arithmetic)."""
    M = np.stack(
        [np.asarray(results[m]["mout"]).astype(np.float32) for m in range(NCORES)]
    )  # [8, 128, 1728]

    dy, dxi, i, j = np.ogrid[0:ND, 0:ND, 0:H, 0:W]
    rp = i & 1
    cp = j & 1
    u = j >> 1
    r2 = i + 2 * dy - 20
    v = u + dxi - 10
    valid = (r2 >= 0) & (r2 < H) & (v >= 0) & (v < 32)
    r2c = np.clip(r2, 0, H - 1)
    vc = np.clip(v, 0, 31)
    k = (r2c - rp) >> 1
    g = k // 4
    rq = k % 4
    h = (g >= 3).astype(int)
    x = np.where(h == 0, g, 5 - g)
    s = (i - rp) >> 1
    l = np.where(h == 0, s, 23 - s)
    off = np.array(SLOT_OFF)[x]
    m = rp * 4 + cp * 2 + h
    part = 32 * rq + vc
    col = off + 32 * l + u
    bm, bp, bc = np.broadcast_arrays(m, part, col)
    out = M[bm, bp, bc]
    out[~np.broadcast_to(valid, out.shape)] = 0.0
    return out.reshape(1, ND * ND, H, W)


def kernel(features_1, features_2):
    nc = _get_program()
    in_maps = _shard_inputs(features_1, features_2)
    res = run_bass_kernel_spmd(nc, in_maps, list(range(NCORES)))
    return _assemble(res.results)


def kernel_traced(features_1, features_2, tmpdir=None):
    """Same as kernel() but with NTFF profiling; returns (output, exec_time_ns)."""
    _ensure_ntff_hook()
    nc = _get_program()
    in_maps = _shard_inputs(features_1, features_2)
    res = run_bass_kernel_spmd(
        nc, in_maps, list(range(NCORES)), trace=True, tmpdir=tmpdir
    )
    return _assemble(res.results), res.exec_time_ns
